# revision 19
# baseline (speedup 1.0000x reference)
"""Supervised-contrastive loss on 8 TRN2 NeuronCores — v2.

Math (matches the reference exactly):
    s_ij   = cosine similarity of feature rows i, j
    E_ij   = exp(s_ij / tau)
    neg_i  = sum_j E_ij * (1 - mask_ij)        (mask = same-class, incl. diag)
    loss   = sum over i and same-class j != i of [ln(E_ij + neg_i) - s_ij/tau] / p_i
             ------------------------------------------------------------
                                  sum_i p_i

Key ideas vs the v1 kernel (80 us):
  * Rows are SORTED BY CLASS on the host, so every same-class pair lies in
    a narrow band around the diagonal.  Each core's 512-row block only needs
    the masked/ln math on a W=768-wide column window instead of all 4096.
  * Each core receives a column-ROTATED copy of fnT so its window is always
    columns [0, W) of its first GEMM chunk -> identical SPMD program on all
    cores, no per-core control flow.
  * fp8 (e4m3) GEMM with DoubleRow perf mode: 2x PE throughput, half the
    DMA bytes.  Downstream exp/ln stay bf16/f32 (modeled rel err ~1.4e-5).
  * One combined ACT function table (exp+ln set) loaded up front manually:
    no 1.3 us table switch between the exp phase and the ln phase.
  * PE warmup matmuls bridge the TRN2 tensor-engine p-state ramp while the
    first DMAs land.
  * ln over the ENTIRE window row with bias=neg: masked entries give
    ln(E+neg), unmasked give ln(neg); host subtracts (W-p_i)*ln(neg_i).

Host (O(N*D) prep/postprocess only): sort, normalize, fp8 quantize,
rotate columns; A_i = lnsum_i - (W - p_i) ln(neg_i); exact-B via class
sums; diagonal correction with the QUANTIZED s_ii; final scalar reduce.
"""

import numpy as np
import ml_dtypes

TAU = 0.1
N, D = 4096, 512
NCORES = 8
ROWS = N // NCORES          # 512 rows per core
ITILES = ROWS // 128        # 4 partition tiles per core
CC = 2                      # column chunks of 2048
KT = D // 128               # 4 contraction sub-tiles of 128
W = 384                     # per-itile masked/ln window (margin 128 each side)
ROT = 128                   # rotated position of the core's own block
N_WARM = 26                 # PE p-state warmup matmuls

_CACHE = {}


def _build_nc():
    import concourse.tile as tile
    import concourse.mybir as mybir
    from concourse import bacc
    from concourse.hw_specs import get_activation_tables

    dt = mybir.dt
    AF = mybir.ActivationFunctionType
    ALU = mybir.AluOpType
    AX = mybir.AxisListType
    PM = mybir.MatmulPerfMode.DoubleRow

    nc = bacc.Bacc(None)
    fnr = nc.declare_dram_parameter("fnr", [D, N], dt.float8e4, isOutput=False)
    tbw = nc.declare_dram_parameter("tbw", [128, 2 * W], dt.bfloat16, isOutput=False)
    tcol = nc.declare_dram_parameter("tcol", [128, ITILES], dt.float32, isOutput=False)
    ln_out = nc.declare_dram_parameter("ln_out", [128, ITILES], dt.float32, isOutput=True)
    neg_out = nc.declare_dram_parameter("neg_out", [128, ITILES], dt.float32, isOutput=True)

    # activation-table set that contains BOTH Exp and Ln
    tables = get_activation_tables(nc.m.arch)
    combo = None
    for i, (name, funcs) in enumerate(tables.items()):
        if AF.Exp in funcs and AF.Ln in funcs:
            combo = i
            break
    assert combo is not None, "no combined exp+ln activation table set"

    with tile.TileContext(nc) as tc:
        with (
            tc.tile_pool(name="persist", bufs=1) as persist,
            tc.tile_pool(name="psum", bufs=2, space="PSUM") as psum,
            tc.tile_pool(name="ebuf", bufs=3) as ebuf,
            tc.tile_pool(name="acc", bufs=1) as accp,
            tc.tile_pool(name="outp", bufs=1) as outp,
            tc.tile_pool(name="lscr", bufs=2) as lpool,
            tc.tile_pool(name="mscr", bufs=2) as mpool,
        ):
            # ---- input DMAs first in program order so the issue queues
            # start immediately (each DIRECT2D trigger costs ~0.6us of
            # sequencer time).  Chunk-0 k-tiles are split (512 | 1536) so
            # the first matmuls' dependencies are tiny transfers.
            fn_sb = []
            for cc in range(CC):
                t_ = persist.tile([128, KT, 2048], dt.float8e4, name=f"fn_{cc}",
                                  tag=f"fn_{cc}")
                fn_sb.append(t_)
            tcol_sb = persist.tile([128, ITILES], dt.float32, tag="tcol")
            tbw_sb = persist.tile([128, 2 * W], dt.bfloat16, tag="tbw")
            with tc.high_priority():
                for k in (0, 1):
                    nc.sync.dma_start(fn_sb[0][:, k, 0:1024],
                                      fnr[k * 128:(k + 1) * 128, 0:1024])
                for k in (2, 3):
                    nc.gpsimd.dma_start(fn_sb[0][:, k, 0:1024],
                                        fnr[k * 128:(k + 1) * 128, 0:1024])
                for k in (0, 1):
                    nc.sync.dma_start(fn_sb[0][:, k, 1024:2048],
                                      fnr[k * 128:(k + 1) * 128, 1024:2048])
                for k in (2, 3):
                    nc.gpsimd.dma_start(fn_sb[0][:, k, 1024:2048],
                                        fnr[k * 128:(k + 1) * 128, 1024:2048])
                nc.gpsimd.dma_start(tcol_sb[:], tcol[:])
                nc.gpsimd.dma_start(tbw_sb[:], tbw[:])
                for k in (0, 1):
                    nc.sync.dma_start(fn_sb[1][:, k, :],
                                      fnr[k * 128:(k + 1) * 128, 2048:4096])
                for k in (2, 3):
                    nc.gpsimd.dma_start(fn_sb[1][:, k, :],
                                        fnr[k * 128:(k + 1) * 128, 2048:4096])

            # ---- combined exp+ln table load on the ACT queue
            nc.scalar.add_instruction(mybir.InstLoadActFuncSet(
                name=nc.get_next_instruction_name(),
                act_func_set_id=combo, ins=[], outs=[]))

            # ---- PE warmup (p-state ramp); one tiny memset, then back-to-back
            # matmuls on the same dummy tile until the input DMAs land.
            wl = persist.tile([128, 2, 128], dt.float8e4, tag="wl")
            nc.vector.memset(wl[:], 0)
            wS = psum.tile([128, 2048], dt.float32, tag="S")
            for _ in range(N_WARM):
                nc.tensor.matmul(
                    wS[:, 0:128], wl[:], wl[:],
                    start=True, stop=True, perf_mode=PM,
                    skip_group_check=True,
                )

            lnout_sb = outp.tile([128, ITILES], dt.float32, tag="lnout")
            negout_sb = outp.tile([128, ITILES], dt.float32, tag="negout")

            rsE_w = [3, 2, 2, 3]    # it0: 2 c0-splits + c1; it3: c0 + 2 c1-splits
            rsE = [accp.tile([128, rsE_w[it]], dt.float32,
                             name=f"rsE_{it}", tag=f"rsE_{it}")
                   for it in range(ITILES)]
            rsEM = [accp.tile([128, 1], dt.float32, name=f"rsEM_{it}",
                              tag=f"rsEM_{it}") for it in range(ITILES)]
            rsT = [accp.tile([128, 1], dt.float32, name=f"rsT_{it}",
                             tag=f"rsT_{it}") for it in range(ITILES)]
            EMs = []

            # ---- phase 1: GEMM + exp with fused row-sum accumulation ----
            for cc in range(CC):
                for it in range(ITILES):
                    E = ebuf.tile([128, 2048], dt.bfloat16, tag="E")
                    if cc == 0 and it == 0:
                        # first chunk: two 1024-wide PSUM halves so the first
                        # exp depends on only 4 matmuls (tile-granular sems)
                        for h in range(2):
                            Sh = psum.tile([128, 1024], dt.float32, tag="S",
                                           name=f"S0{h}")
                            for kp in range(2):
                                for nb in range(2):
                                    c0 = h * 1024 + nb * 512
                                    nc.tensor.matmul(
                                        Sh[:, nb * 512:(nb + 1) * 512],
                                        fn_sb[0][:, 2 * kp:2 * kp + 2, ROT:ROT + 128],
                                        fn_sb[0][:, 2 * kp:2 * kp + 2, c0:c0 + 512],
                                        start=(kp == 0), stop=(kp == 1),
                                        perf_mode=PM, skip_group_check=True,
                                    )
                            nc.scalar.activation(
                                E[:, h * 1024:(h + 1) * 1024], Sh[:], AF.Exp,
                                scale=1.0 / TAU, accum_out=rsE[0][:, h:h + 1])
                        S = None
                    else:
                        S = psum.tile([128, 2048], dt.float32, tag="S")
                        for kp in range(2):
                            for nb in range(4):
                                nc.tensor.matmul(
                                    S[:, nb * 512:(nb + 1) * 512],
                                    fn_sb[0][:, 2 * kp:2 * kp + 2,
                                             ROT + it * 128:ROT + (it + 1) * 128],
                                    fn_sb[cc][:, 2 * kp:2 * kp + 2, nb * 512:(nb + 1) * 512],
                                    start=(kp == 0), stop=(kp == 1),
                                    perf_mode=PM, skip_group_check=True,
                                )
                    if cc == 0 and it == 0:
                        pass
                    elif cc == 1 and it == 3:
                        # split the very last exp to shorten the output tail
                        nc.scalar.activation(E[:, 0:1024], S[:, 0:1024], AF.Exp,
                                             scale=1.0 / TAU, accum_out=rsE[3][:, 1:2])
                        nc.scalar.activation(E[:, 1024:2048], S[:, 1024:2048], AF.Exp,
                                             scale=1.0 / TAU, accum_out=rsE[3][:, 2:3])
                    else:
                        col = (0 if cc == 0 else 1) if it != 0 else 2
                        nc.scalar.activation(E[:], S[:], AF.Exp,
                                             scale=1.0 / TAU,
                                             accum_out=rsE[it][:, col:col + 1])
                    if cc == 0:
                        # itile it's class window: rotated cols [128it, 128it+W)
                        EM = persist.tile([128, W], dt.bfloat16, name=f"em_{it}",
                                         tag=f"em_{it}")
                        nc.vector.scalar_tensor_tensor(
                            EM[:], tbw_sb[:, 128 * it:128 * it + W],
                            tcol_sb[:, it:it + 1], E[:, 128 * it:128 * it + W],
                            ALU.is_equal, ALU.mult,
                            accum_out=rsEM[it][:],
                        )
                        EMs.append(EM)
                    else:
                        nc.vector.tensor_reduce(rsT[it][:], rsE[it][:], AX.X, ALU.add)
                        nc.vector.tensor_sub(negout_sb[:, it:it + 1],
                                             rsT[it][:], rsEM[it][:])

            nc.gpsimd.dma_start(neg_out[:], negout_sb[:])

            # ---- phase 2: ln over the window, same ACT table ----
            for it in range(ITILES):
                L = lpool.tile([128, W], dt.bfloat16, tag="L")
                nc.scalar.activation(
                    L[:], EMs[it][:], AF.Ln,
                    bias=negout_sb[:, it:it + 1], scale=1.0,
                    accum_out=lnout_sb[:, it:it + 1],
                )
                if it == 2:
                    nc.scalar.dma_start(ln_out[:, 0:3], lnout_sb[:, 0:3])
            nc.scalar.dma_start(ln_out[:, 3:4], lnout_sb[:, 3:4])

    nc.finalize()
    return nc


def _get_nc():
    if "nc" not in _CACHE:
        _CACHE["nc"] = _build_nc()
    return _CACHE["nc"]


def _host_prep(features, targets):
    bf16 = ml_dtypes.bfloat16
    e4m3 = ml_dtypes.float8_e4m3
    f = np.asarray(features, np.float32)
    t = np.asarray(targets).astype(np.int64)

    perm = np.argsort(t, kind="stable")
    ts = t[perm]
    fs = f[perm]

    nrm = np.sqrt((fs.astype(np.float64) ** 2).sum(1))
    nrm = np.where(nrm == 0, 1e-8, nrm)
    fn = (fs * (1.0 / nrm)[:, None].astype(np.float32)).astype(np.float32)
    fnq = fn.astype(e4m3)                       # what the device GEMM sees
    fnT8 = np.ascontiguousarray(fnq.T)          # [D, N] fp8

    ts_b = ts.astype(np.float32).astype(bf16)
    in_maps = []
    for c in range(NCORES):
        wc = (512 * c - ROT) % N
        idx = (wc + np.arange(N)) % N
        # window-coverage check: every same-class column of itile it's rows
        # must land in rotated positions [128it, 128it+W)
        for it in range(ITILES):
            rows = ts[c * 512 + it * 128:c * 512 + (it + 1) * 128]
            lo, hi = np.searchsorted(ts, [rows[0], rows[-1] + 1])
            rlo, rhi = (lo - wc) % N, (hi - 1 - wc) % N
            assert 128 * it <= rlo and rhi < 128 * it + W, (
                f"window violated core {c} it {it}: [{rlo},{rhi}] "
                f"vs [{128 * it},{128 * it + W})"
            )
        in_maps.append({
            "fnr": np.ascontiguousarray(fnT8[:, idx]),
            "tbw": np.ascontiguousarray(
                np.broadcast_to(ts_b[idx[:2 * W]][None, :], (128, 2 * W))),
            "tcol": np.ascontiguousarray(
                ts[c * 512:(c + 1) * 512].astype(np.float32)
                .reshape(ITILES, 128).T),
        })
    bundle = {"fn": fn, "fnq": fnq.astype(np.float32), "ts": ts}
    return bundle, t, in_maps


def _host_post(bundle, lnsum_rows, neg_rows):
    fn = bundle["fn"].astype(np.float64)
    fnq = bundle["fnq"].astype(np.float64)
    ts = bundle["ts"]
    p = np.bincount(ts)[ts].astype(np.float64)
    A = lnsum_rows - (W - p) * np.log(neg_rows)
    g = np.zeros((int(ts.max()) + 1, D), np.float64)
    np.add.at(g, ts, fn)
    B = (fn * g[ts]).sum(1) / TAU
    sqii = (fnq ** 2).sum(1)
    corr = np.log(np.exp(sqii / TAU) + neg_rows) - 1.0 / TAU
    numer = A - B - corr
    loss = (numer / p).sum() / p.sum()
    return np.float32(loss)


def _rows_from_out(per_core_outs, key):
    # [128, ITILES] per core, row index = core*512 + it*128 + part
    rows = np.empty(N, np.float64)
    for c, out in enumerate(per_core_outs):
        arr = np.asarray(out[key], np.float64)  # [128, ITILES]
        rows[c * ROWS:(c + 1) * ROWS] = arr.T.reshape(ROWS)
    return rows


def _run(in_maps, trace=False):
    from concourse.bass_utils import run_bass_kernel_spmd
    nc = _get_nc()
    res = run_bass_kernel_spmd(
        nc, in_maps, core_ids=list(range(NCORES)), trace=trace,
    )
    return res


def kernel(features, targets):
    bundle, t, in_maps = _host_prep(features, targets)
    res = _run(in_maps, trace=False)
    lnsum_rows = _rows_from_out(res.results, "ln_out")
    neg_rows = _rows_from_out(res.results, "neg_out")
    return _host_post(bundle, lnsum_rows, neg_rows)


# revision 20
# speedup vs baseline: 1.0959x; 1.0959x over previous
"""Supervised-contrastive loss on 8 TRN2 NeuronCores — v2.

Math (matches the reference exactly):
    s_ij   = cosine similarity of feature rows i, j
    E_ij   = exp(s_ij / tau)
    neg_i  = sum_j E_ij * (1 - mask_ij)        (mask = same-class, incl. diag)
    loss   = sum over i and same-class j != i of [ln(E_ij + neg_i) - s_ij/tau] / p_i
             ------------------------------------------------------------
                                  sum_i p_i

Key ideas vs the v1 kernel (80 us):
  * Rows are SORTED BY CLASS on the host, so every same-class pair lies in
    a narrow band around the diagonal.  Each core's 512-row block only needs
    the masked/ln math on a W=768-wide column window instead of all 4096.
  * Each core receives a column-ROTATED copy of fnT so its window is always
    columns [0, W) of its first GEMM chunk -> identical SPMD program on all
    cores, no per-core control flow.
  * fp8 (e4m3) GEMM with DoubleRow perf mode: 2x PE throughput, half the
    DMA bytes.  Downstream exp/ln stay bf16/f32 (modeled rel err ~1.4e-5).
  * One combined ACT function table (exp+ln set) loaded up front manually:
    no 1.3 us table switch between the exp phase and the ln phase.
  * PE warmup matmuls bridge the TRN2 tensor-engine p-state ramp while the
    first DMAs land.
  * ln over the ENTIRE window row with bias=neg: masked entries give
    ln(E+neg), unmasked give ln(neg); host subtracts (W-p_i)*ln(neg_i).

Host (O(N*D) prep/postprocess only): sort, normalize, fp8 quantize,
rotate columns; A_i = lnsum_i - (W - p_i) ln(neg_i); exact-B via class
sums; diagonal correction with the QUANTIZED s_ii; final scalar reduce.
"""

import numpy as np
import ml_dtypes

TAU = 0.1
N, D = 4096, 512
NCORES = 8
ROWS = N // NCORES          # 512 rows per core
ITILES = ROWS // 128        # 4 partition tiles per core
CC = 2                      # column chunks of 2048
KT = D // 128               # 4 contraction sub-tiles of 128
W = 384                     # per-itile masked/ln window (margin 128 each side)
ROT = 128                   # rotated position of the core's own block
N_WARM = 30                 # PE p-state warmup matmuls

_CACHE = {}


def _build_nc():
    import concourse.tile as tile
    import concourse.mybir as mybir
    from concourse import bacc
    from concourse.hw_specs import get_activation_tables

    dt = mybir.dt
    AF = mybir.ActivationFunctionType
    ALU = mybir.AluOpType
    AX = mybir.AxisListType
    PM = mybir.MatmulPerfMode.DoubleRow

    nc = bacc.Bacc(None)
    fnr = nc.declare_dram_parameter("fnr", [D, N], dt.float8e4, isOutput=False)
    tbw = nc.declare_dram_parameter("tbw", [128, 2 * W], dt.bfloat16, isOutput=False)
    tcol = nc.declare_dram_parameter("tcol", [128, ITILES], dt.float32, isOutput=False)
    ln_out = nc.declare_dram_parameter("ln_out", [128, ITILES], dt.float32, isOutput=True)
    neg_out = nc.declare_dram_parameter("neg_out", [128, ITILES], dt.float32, isOutput=True)

    # activation-table set that contains BOTH Exp and Ln
    tables = get_activation_tables(nc.m.arch)
    combo = None
    for i, (name, funcs) in enumerate(tables.items()):
        if AF.Exp in funcs and AF.Ln in funcs:
            combo = i
            break
    assert combo is not None, "no combined exp+ln activation table set"

    with tile.TileContext(nc) as tc:
        with (
            tc.tile_pool(name="persist", bufs=1) as persist,
            tc.tile_pool(name="psum", bufs=2, space="PSUM") as psum,
            tc.tile_pool(name="ebuf", bufs=3) as ebuf,
            tc.tile_pool(name="acc", bufs=1) as accp,
            tc.tile_pool(name="outp", bufs=1) as outp,
            tc.tile_pool(name="lscr", bufs=2) as lpool,
            tc.tile_pool(name="mscr", bufs=2) as mpool,
        ):
            # ---- input DMAs first in program order so the issue queues
            # start immediately (each DIRECT2D trigger costs ~0.6us of
            # sequencer time).  Chunk-0 k-tiles are split (512 | 1536) so
            # the first matmuls' dependencies are tiny transfers.
            fn_sb = []
            for cc in range(CC):
                t_ = persist.tile([128, KT, 2048], dt.float8e4, name=f"fn_{cc}",
                                  tag=f"fn_{cc}")
                fn_sb.append(t_)
            tcol_sb = persist.tile([128, ITILES], dt.float32, tag="tcol")
            tbw_sb = persist.tile([128, 2 * W], dt.bfloat16, tag="tbw")
            with tc.high_priority():
                for k in (0, 1):
                    nc.sync.dma_start(fn_sb[0][:, k, 0:1024],
                                      fnr[k * 128:(k + 1) * 128, 0:1024])
                for k in (2, 3):
                    nc.gpsimd.dma_start(fn_sb[0][:, k, 0:1024],
                                        fnr[k * 128:(k + 1) * 128, 0:1024])
                for k in (0, 1):
                    nc.sync.dma_start(fn_sb[0][:, k, 1024:2048],
                                      fnr[k * 128:(k + 1) * 128, 1024:2048])
                for k in (2, 3):
                    nc.gpsimd.dma_start(fn_sb[0][:, k, 1024:2048],
                                        fnr[k * 128:(k + 1) * 128, 1024:2048])
                nc.gpsimd.dma_start(tcol_sb[:], tcol[:])
                nc.gpsimd.dma_start(tbw_sb[:], tbw[:])
                for k in (0, 1):
                    nc.sync.dma_start(fn_sb[1][:, k, :],
                                      fnr[k * 128:(k + 1) * 128, 2048:4096])
                for k in (2, 3):
                    nc.gpsimd.dma_start(fn_sb[1][:, k, :],
                                        fnr[k * 128:(k + 1) * 128, 2048:4096])

            # ---- combined exp+ln table load on the ACT queue
            nc.scalar.add_instruction(mybir.InstLoadActFuncSet(
                name=nc.get_next_instruction_name(),
                act_func_set_id=combo, ins=[], outs=[]))

            # ---- PE warmup (p-state ramp); one tiny memset, then back-to-back
            # matmuls on the same dummy tile until the input DMAs land.
            wl = persist.tile([128, 2, 128], dt.float8e4, tag="wl")
            nc.vector.memset(wl[:], 0)
            wS = psum.tile([128, 2048], dt.float32, tag="S")
            for _ in range(N_WARM):
                nc.tensor.matmul(
                    wS[:, 0:128], wl[:], wl[:],
                    start=True, stop=True, perf_mode=PM,
                    skip_group_check=True,
                )

            lnout_sb = outp.tile([128, ITILES], dt.float32, tag="lnout")
            negout_sb = outp.tile([128, ITILES], dt.float32, tag="negout")

            rsE_w = [3, 2, 2, 3]    # it0: 2 c0-splits + c1; it3: c0 + 2 c1-splits
            rsE = [accp.tile([128, rsE_w[it]], dt.float32,
                             name=f"rsE_{it}", tag=f"rsE_{it}")
                   for it in range(ITILES)]
            rsEM = [accp.tile([128, 1], dt.float32, name=f"rsEM_{it}",
                              tag=f"rsEM_{it}") for it in range(ITILES)]
            rsT = [accp.tile([128, 1], dt.float32, name=f"rsT_{it}",
                             tag=f"rsT_{it}") for it in range(ITILES)]
            EMs = []

            # ---- phase 1: GEMM + exp with fused row-sum accumulation ----
            for cc in range(CC):
                for it in range(ITILES):
                    E = ebuf.tile([128, 2048], dt.bfloat16, tag="E")
                    if cc == 0 and it == 0:
                        # first chunk: two 1024-wide PSUM halves so the first
                        # exp depends on only 4 matmuls (tile-granular sems)
                        for h in range(2):
                            Sh = psum.tile([128, 1024], dt.float32, tag="S",
                                           name=f"S0{h}")
                            for kp in range(2):
                                for nb in range(2):
                                    c0 = h * 1024 + nb * 512
                                    nc.tensor.matmul(
                                        Sh[:, nb * 512:(nb + 1) * 512],
                                        fn_sb[0][:, 2 * kp:2 * kp + 2, ROT:ROT + 128],
                                        fn_sb[0][:, 2 * kp:2 * kp + 2, c0:c0 + 512],
                                        start=(kp == 0), stop=(kp == 1),
                                        perf_mode=PM, skip_group_check=True,
                                    )
                            nc.scalar.activation(
                                E[:, h * 1024:(h + 1) * 1024], Sh[:], AF.Exp,
                                scale=1.0 / TAU, accum_out=rsE[0][:, h:h + 1])
                        S = None
                    else:
                        S = psum.tile([128, 2048], dt.float32, tag="S")
                        for kp in range(2):
                            for nb in range(4):
                                nc.tensor.matmul(
                                    S[:, nb * 512:(nb + 1) * 512],
                                    fn_sb[0][:, 2 * kp:2 * kp + 2,
                                             ROT + it * 128:ROT + (it + 1) * 128],
                                    fn_sb[cc][:, 2 * kp:2 * kp + 2, nb * 512:(nb + 1) * 512],
                                    start=(kp == 0), stop=(kp == 1),
                                    perf_mode=PM, skip_group_check=True,
                                )
                    if cc == 0 and it == 0:
                        pass
                    elif cc == 1 and it == 3:
                        # split the very last exp to shorten the output tail
                        nc.scalar.activation(E[:, 0:1024], S[:, 0:1024], AF.Exp,
                                             scale=1.0 / TAU, accum_out=rsE[3][:, 1:2])
                        nc.scalar.activation(E[:, 1024:2048], S[:, 1024:2048], AF.Exp,
                                             scale=1.0 / TAU, accum_out=rsE[3][:, 2:3])
                    else:
                        col = (0 if cc == 0 else 1) if it != 0 else 2
                        nc.scalar.activation(E[:], S[:], AF.Exp,
                                             scale=1.0 / TAU,
                                             accum_out=rsE[it][:, col:col + 1])
                    if cc == 0:
                        # itile it's class window: rotated cols [128it, 128it+W)
                        EM = persist.tile([128, W], dt.bfloat16, name=f"em_{it}",
                                         tag=f"em_{it}")
                        nc.vector.scalar_tensor_tensor(
                            EM[:], tbw_sb[:, 128 * it:128 * it + W],
                            tcol_sb[:, it:it + 1], E[:, 128 * it:128 * it + W],
                            ALU.is_equal, ALU.mult,
                            accum_out=rsEM[it][:],
                        )
                        EMs.append(EM)
                    else:
                        nc.vector.tensor_reduce(rsT[it][:], rsE[it][:], AX.X, ALU.add)
                        nc.vector.tensor_sub(negout_sb[:, it:it + 1],
                                             rsT[it][:], rsEM[it][:])

            nc.gpsimd.dma_start(neg_out[:], negout_sb[:])

            # ---- phase 2: ln over the window, same ACT table ----
            for it in range(ITILES):
                L = lpool.tile([128, W], dt.bfloat16, tag="L")
                nc.scalar.activation(
                    L[:], EMs[it][:], AF.Ln,
                    bias=negout_sb[:, it:it + 1], scale=1.0,
                    accum_out=lnout_sb[:, it:it + 1],
                )
                if it == 2:
                    nc.scalar.dma_start(ln_out[:, 0:3], lnout_sb[:, 0:3])
            nc.scalar.dma_start(ln_out[:, 3:4], lnout_sb[:, 3:4])

    nc.finalize()
    return nc


def _get_nc():
    if "nc" not in _CACHE:
        _CACHE["nc"] = _build_nc()
    return _CACHE["nc"]


def _host_prep(features, targets):
    bf16 = ml_dtypes.bfloat16
    e4m3 = ml_dtypes.float8_e4m3
    f = np.asarray(features, np.float32)
    t = np.asarray(targets).astype(np.int64)

    perm = np.argsort(t, kind="stable")
    ts = t[perm]
    fs = f[perm]

    nrm = np.sqrt((fs.astype(np.float64) ** 2).sum(1))
    nrm = np.where(nrm == 0, 1e-8, nrm)
    fn = (fs * (1.0 / nrm)[:, None].astype(np.float32)).astype(np.float32)
    fnq = fn.astype(e4m3)                       # what the device GEMM sees
    fnT8 = np.ascontiguousarray(fnq.T)          # [D, N] fp8

    ts_b = ts.astype(np.float32).astype(bf16)
    in_maps = []
    for c in range(NCORES):
        wc = (512 * c - ROT) % N
        idx = (wc + np.arange(N)) % N
        # window-coverage check: every same-class column of itile it's rows
        # must land in rotated positions [128it, 128it+W)
        for it in range(ITILES):
            rows = ts[c * 512 + it * 128:c * 512 + (it + 1) * 128]
            lo, hi = np.searchsorted(ts, [rows[0], rows[-1] + 1])
            rlo, rhi = (lo - wc) % N, (hi - 1 - wc) % N
            assert 128 * it <= rlo and rhi < 128 * it + W, (
                f"window violated core {c} it {it}: [{rlo},{rhi}] "
                f"vs [{128 * it},{128 * it + W})"
            )
        in_maps.append({
            "fnr": np.ascontiguousarray(fnT8[:, idx]),
            "tbw": np.ascontiguousarray(
                np.broadcast_to(ts_b[idx[:2 * W]][None, :], (128, 2 * W))),
            "tcol": np.ascontiguousarray(
                ts[c * 512:(c + 1) * 512].astype(np.float32)
                .reshape(ITILES, 128).T),
        })
    bundle = {"fn": fn, "fnq": fnq.astype(np.float32), "ts": ts}
    return bundle, t, in_maps


def _host_post(bundle, lnsum_rows, neg_rows):
    fn = bundle["fn"].astype(np.float64)
    fnq = bundle["fnq"].astype(np.float64)
    ts = bundle["ts"]
    p = np.bincount(ts)[ts].astype(np.float64)
    A = lnsum_rows - (W - p) * np.log(neg_rows)
    g = np.zeros((int(ts.max()) + 1, D), np.float64)
    np.add.at(g, ts, fn)
    B = (fn * g[ts]).sum(1) / TAU
    sqii = (fnq ** 2).sum(1)
    corr = np.log(np.exp(sqii / TAU) + neg_rows) - 1.0 / TAU
    numer = A - B - corr
    loss = (numer / p).sum() / p.sum()
    return np.float32(loss)


def _rows_from_out(per_core_outs, key):
    # [128, ITILES] per core, row index = core*512 + it*128 + part
    rows = np.empty(N, np.float64)
    for c, out in enumerate(per_core_outs):
        arr = np.asarray(out[key], np.float64)  # [128, ITILES]
        rows[c * ROWS:(c + 1) * ROWS] = arr.T.reshape(ROWS)
    return rows


def _run(in_maps, trace=False):
    from concourse.bass_utils import run_bass_kernel_spmd
    nc = _get_nc()
    res = run_bass_kernel_spmd(
        nc, in_maps, core_ids=list(range(NCORES)), trace=trace,
    )
    return res


def kernel(features, targets):
    bundle, t, in_maps = _host_prep(features, targets)
    res = _run(in_maps, trace=False)
    lnsum_rows = _rows_from_out(res.results, "ln_out")
    neg_rows = _rows_from_out(res.results, "neg_out")
    return _host_post(bundle, lnsum_rows, neg_rows)
